# revision 36
# baseline (speedup 1.0000x reference)
"""Trainium2 Bass kernel for a transformer encoder layer (B=2, S=2048,
D=1024, H=16, FFN=4096), sharded over 8 NeuronCores.

Sharding: token-parallel. Cores 0-3 process batch 0, cores 4-7 batch 1;
each core owns a 512-token query window and computes the full layer for
those tokens. K/V are computed per-core for the whole batch (duplicated
across the 4 cores of a batch group) - no collectives.

Layout: activations are feature-major ([d, token]) so all matmuls chain
without transposes. Attention scores are computed transposed ([kv, q]);
softmax runs without max-subtraction (pad keys get a -30000 bias so exp
underflows to exactly 0). The denominator comes from an appended
ones-column in V; per-query normalization uses reciprocal_approx_fast on
the two [1,512] denominator rows in PSUM, then one K=1 PE matmul
broadcasts both heads' reciprocals across partitions.

Pipelining: the kernel is software-pipelined by issue order. K^T and V
for head-half 1 are issued as "filler" matmul chains interleaved into
attention pairs 0-4, so the PE never idles while the Scalar engine
computes exps. Weight/activation DMAs are batched ([128, k, w] 3D
transfers) to cut SP-engine trigger serialization. LayerNorm sum-matmuls
interleave into the Wo / FFN2 loops; LN tails use approx reciprocals and
split work across DVE/Pool/Scalar.

Masked keys are compacted away on the host: only unmasked positions are
projected/attended (~half of S). Matmul chain runs in bf16; residual
adds, layernorm statistics and softmax denominators stay in fp32.
"""

from collections import deque
from contextlib import ExitStack

import ml_dtypes
import numpy as np

import concourse.bass as bass  # noqa: F401
import concourse.mybir as mybir
import concourse.tile as tile
from concourse import bacc
from concourse.bass_utils import run_bass_kernel_spmd

f32 = mybir.dt.float32
f32r = mybir.dt.float32r
bf16 = mybir.dt.bfloat16
AF = mybir.ActivationFunctionType
ALU = mybir.AluOpType

D = 1024
H = 16
DEP = 64
HID = 4096
B = 2
S = 2048
QLOC = 512
NCORES = 8
EPS = 1e-6
PADBIAS = -30000.0

P = 128
KT_D = D // P      # 8
MT_D = D // P      # 8
MT_H = HID // P    # 32
NPAIR = H // 2     # 8
VW = DEP + 1       # 65

# packed bias/affine column offsets in cb ([P, 96])
CB_OFF = {"bq": 0, "bk": 8, "bo": 16, "b1": 24, "b2": 56,
          "a1": 64, "be1": 72, "a2": 80, "be2": 88}

PHASES = {}


def _mark(nc, name):
    PHASES[name] = nc.next_id()


def _chunks(total, maxc):
    n = (total + maxc - 1) // maxc
    base = (total // n + 127) // 128 * 128
    out = []
    off = 0
    while off < total:
        c = min(base, total - off)
        out.append((off, c))
        off += c
    return out


def build(nkv: int, dbg: str | None = None):
    assert nkv % P == 0
    nkt = nkv // P

    nc = bacc.Bacc(None, target_bir_lowering=False, debug=False)

    xq_d = nc.dram_tensor("xq", [D, QLOC], bf16, kind="ExternalInput")
    xqf_d = nc.dram_tensor("xqf", [D, QLOC], f32, kind="ExternalInput")
    xkv_d = nc.dram_tensor("xkv", [D, nkv], bf16, kind="ExternalInput")
    mb_d = nc.dram_tensor("mb", [P, nkt], f32, kind="ExternalInput")
    wq_d = nc.dram_tensor("wq", [D, D], bf16, kind="ExternalInput")
    wk_d = nc.dram_tensor("wk", [D, D], bf16, kind="ExternalInput")
    wv_d = nc.dram_tensor("wv", [D, D], bf16, kind="ExternalInput")
    wo_d = nc.dram_tensor("wo", [D, D], bf16, kind="ExternalInput")
    w1_d = nc.dram_tensor("w1", [D, HID], bf16, kind="ExternalInput")
    w2_d = nc.dram_tensor("w2", [HID, D], bf16, kind="ExternalInput")
    cb_d = nc.dram_tensor("cb", [P, 96], f32, kind="ExternalInput")
    cone1_d = nc.dram_tensor("cone1", [P, 1], f32, kind="ExternalInput")
    crow_d = nc.dram_tensor("crow", [1, P], f32, kind="ExternalInput")
    out_d = nc.dram_tensor("out", [D, QLOC], f32, kind="ExternalOutput")

    kv_chunks = _chunks(nkv, 512)

    def dram3(dt_, r0, nrt, c0, cw):
        """[128, nrt, cw] view of dram rows r0..r0+128*nrt, cols c0..c0+cw."""
        return dt_[r0:r0 + P * nrt, c0:c0 + cw].rearrange(
            "(k p) c -> p k c", p=P)

    def dbg_dump(nc_, tiles, cst_pool):
        """DMA up to 8 [P, QLOC]-ish tiles to out_d (debug)."""
        for m, t in enumerate(tiles[:MT_D]):
            if t.dtype == f32:
                src = t[:]
            elif t.dtype == f32r:
                src = t[:].bitcast(f32)
            else:
                tmp = cst_pool.tile([P, QLOC], f32, name=f"dbgf{m}",
                                    tag="dbgf", bufs=2)
                nc_.scalar.copy(tmp[:], t[:])
                src = tmp[:]
            nc_.sync.dma_start(out=out_d[m * P:(m + 1) * P, :], in_=src)

    with tile.TileContext(nc) as tc, \
         nc.allow_low_precision(reason="bf16/f32r matmul inputs"), \
         ExitStack() as ctx:
        # ================= pool layout (LIFO per side) =================
        # LEFT:  cst, p_qr, p_kt, p_o1b | p_attnT | p_vaug, epl, nrm
        #        (pop: nrm, epl, vaug after attn; attnT after wo)
        #        then p_ht | w1p (pop) | w2p (pop)
        # RIGHT: p_xq, ln_s | wqp, wkp, wvp, p_xkv, psm(PSUM)
        #        (pop psm, xkv, wvp, wkp, wqp after attn)
        #        then wop, pp2(PSUM), lnp1(PSUM)
        #        (pop lnp1 after ln1, pp2+wop after w1)
        #        then lnp2(PSUM), fpp(PSUM) (pop fpp, lnp2)
        cst = ctx.enter_context(tc.tile_pool(name="cst", bufs=1))
        p_qr = ctx.enter_context(tc.tile_pool(name="p_qr", bufs=MT_D))
        p_kt = ctx.enter_context(tc.tile_pool(name="p_kt", bufs=MT_D))
        p_o1b = ctx.enter_context(tc.tile_pool(name="p_o1b", bufs=1))
        es_attnT = ExitStack()
        p_attnT = es_attnT.enter_context(
            tc.tile_pool(name="p_attnT", bufs=MT_D))
        es_vaug = ExitStack()
        p_vaug = es_vaug.enter_context(tc.tile_pool(name="p_vaug", bufs=nkt))
        ep = ExitStack()
        epl = ep.enter_context(tc.tile_pool(name="epl", bufs=2))
        nrm = ep.enter_context(tc.tile_pool(name="nrm", bufs=2))

        p_xq = ctx.enter_context(tc.tile_pool(name="p_xq", bufs=1,
                                              side="right"))
        ln_s = ctx.enter_context(tc.tile_pool(name="ln_s", bufs=2,
                                              side="right"))
        es_wq = ExitStack()
        wqp = es_wq.enter_context(tc.tile_pool(name="wqp", bufs=1,
                                               side="right"))
        es_wk = ExitStack()
        wkp = es_wk.enter_context(tc.tile_pool(name="wkp", bufs=1,
                                               side="right"))
        es_wv = ExitStack()
        wvp = es_wv.enter_context(tc.tile_pool(name="wvp", bufs=1,
                                               side="right"))
        es_kvx = ExitStack()
        p_xkv = es_kvx.enter_context(tc.tile_pool(name="p_xkv", bufs=1,
                                                  side="right"))
        es_ps = ExitStack()
        psm = es_ps.enter_context(
            tc.tile_pool(name="psm", bufs=2, space="PSUM", side="right"))

        # ================= DMAs (priority order) =================
        # small constants first: qt/kt drains need cb; cheap triggers
        cb_sb = cst.tile([P, 96], f32)
        nc.sync.dma_start(out=cb_sb[:], in_=cb_d[:])
        ones = cst.tile([P, 1], f32r)
        nc.sync.dma_start(out=ones[:], in_=cone1_d[:].bitcast(f32r))
        onesr = cst.tile([1, P], f32r)
        nc.sync.dma_start(out=onesr[:], in_=crow_d[:].bitcast(f32r))
        mbias = cst.tile([P, nkt], f32)
        nc.sync.dma_start(out=mbias[:], in_=mb_d[:])

        def col(nm, i):
            o = CB_OFF[nm] + i
            return cb_sb[:, o:o + 1]

        xq_sb = p_xq.tile([P, KT_D * QLOC], bf16)
        nc.sync.dma_start(
            out=xq_sb[:].rearrange("p (k c) -> p k c", c=QLOC),
            in_=dram3(xq_d, 0, KT_D, 0, QLOC))
        wq_sb = wqp.tile([P, 2 * KT_D * 512], bf16)
        for h in range(2):
            nc.sync.dma_start(
                out=wq_sb[:, h * 4096:(h + 1) * 4096].rearrange(
                    "p (k c) -> p k c", c=512),
                in_=dram3(wq_d, 0, KT_D, h * 512, 512))
        xkv_sb = p_xkv.tile([P, KT_D * nkv], bf16)
        for h in range(2):
            nc.sync.dma_start(
                out=xkv_sb[:, h * 4 * nkv:(h + 1) * 4 * nkv].rearrange(
                    "p (k c) -> p k c", c=nkv),
                in_=dram3(xkv_d, h * 512, 4, 0, nkv))
        wk_sb = wkp.tile([P, 2 * KT_D * 512], bf16)
        for h in range(2):
            nc.scalar.dma_start(
                out=wk_sb[:, h * 4096:(h + 1) * 4096].rearrange(
                    "p (k c) -> p k c", c=512),
                in_=dram3(wk_d, 0, KT_D, h * 512, 512))
        wv_sb = wvp.tile([P, 2 * KT_D * 512], bf16)
        for h in range(2):
            nc.scalar.dma_start(
                out=wv_sb[:, h * 4096:(h + 1) * 4096].rearrange(
                    "p (k c) -> p k c", c=512),
                in_=dram3(wv_d, 0, KT_D, h * 512, 512))

        # ================= persistent activation tiles =================
        vaug = []
        for ti in range(nkt):
            t = p_vaug.tile([P, H * VW], bf16, name=f"vaug{ti}", tag="vaug")
            v3 = t[:].rearrange("p (h c) -> p h c", c=VW)
            nc.gpsimd.memset(v3[:, :, DEP], 1.0)
            vaug.append(t)
        kt = [p_kt.tile([P, nkv], bf16, name=f"kt{m}", tag="kt")
              for m in range(NPAIR)]

        # ---- Q^T (half 0 now; half 1 as attention fillers) ----
        _mark(nc, 'qt')
        qt = [None] * MT_D

        def qt_chain(m):
            h, ml = m // 4, m % 4
            psw = psm.tile([P, 2 * QLOC], f32, name="qt_ps", tag="sc")
            ps = psw[:, 0:QLOC]
            for k in range(KT_D):
                nc.tensor.matmul(
                    ps, wq_sb[:, h * 4096 + k * 512 + ml * 128:
                              h * 4096 + k * 512 + (ml + 1) * 128],
                    xq_sb[:, k * QLOC:(k + 1) * QLOC],
                    start=(k == 0), stop=(k == KT_D - 1))
            t = p_qr.tile([P, QLOC], bf16, name=f"qt{m}", tag="qr")
            nc.scalar.activation(t[:], ps, AF.Identity, bias=col("bq", m))
            qt[m] = t

        for m in range(4):
            qt_chain(m)
        if dbg == 'qt':
            dbg_dump(nc, [t[:] for t in qt], cst)

        # ---- chain builders ----
        def kt_chain(m, off, cw):
            h, ml = m // 4, m % 4
            psw = psm.tile([P, 2 * QLOC], f32, name="kt_ps", tag="sc")
            ps = psw[:, 0:512]
            for k in range(KT_D):
                nc.tensor.matmul(
                    ps[:, :cw], wk_sb[:, h * 4096 + k * 512 + ml * 128:
                                      h * 4096 + k * 512 + (ml + 1) * 128],
                    xkv_sb[:, k * nkv + off:k * nkv + off + cw],
                    start=(k == 0), stop=(k == KT_D - 1))
            nc.vector.tensor_scalar_add(kt[m][:, off:off + cw], ps[:, :cw],
                                        col("bk", m))

        def v_chain(h, ti):
            psw = psm.tile([P, 2 * QLOC], f32, name="v_ps", tag="sc")
            ps = psw[:, 0:512]
            for k in range(KT_D):
                nc.tensor.matmul(
                    ps, xkv_sb[:, k * nkv + ti * P:k * nkv + (ti + 1) * P],
                    wv_sb[:, h * 4096 + k * 512:h * 4096 + (k + 1) * 512],
                    start=(k == 0), stop=(k == KT_D - 1))
            v3 = vaug[ti][:].rearrange("p (h c) -> p h c", c=VW)
            vsrc = ps.rearrange("p (h c) -> p h c", c=DEP)
            nc.vector.tensor_copy(v3[:, h * 8:(h + 1) * 8, 0:DEP], vsrc)

        # ---- K^T / V half 0 ----
        _mark(nc, 'kt')
        for m in range(4):
            for off, cw in kv_chunks:
                kt_chain(m, off, cw)
        _mark(nc, 'v')
        for ti in range(nkt):
            v_chain(0, ti)

        # fillers: half-1 Q/K/V chains interleaved into attention pairs.
        # Constraints: kt[m]/qt[m] before pair m//? uses them; v(1,ti)
        # before AV(pair4, ti). Spread to keep the PE fed while Scalar
        # does exps.
        pair_fill = {p: deque() for p in range(NPAIR)}
        items = deque()
        for m in range(4, 8):
            for off, cw in kv_chunks:
                items.append((kt_chain, (m, off, cw)))
        for ti in range(nkt // 2):
            items.append((v_chain, (1, ti)))
        for i, it in enumerate(items):          # round-robin pairs 0-3
            pair_fill[i % 4].append(it)
        pair_fill[3].append((qt_chain, (4,)))
        vfill_p4 = deque()   # second half of V -> step-aligned in pair 4
        for ti in range(nkt // 2, nkt):
            vfill_p4.append((v_chain, (1, ti)))
        if NPAIR >= 6:
            pair_fill[4].append((qt_chain, (5,)))
            pair_fill[5].append((qt_chain, (6,)))
            pair_fill[6].append((qt_chain, (7,)))

        # ---- attention ----
        _mark(nc, 'attn')
        den = nrm.tile([VW, 2 * QLOC], f32, name="den", tag="den", bufs=1)
        rec = nrm.tile([VW, 2 * QLOC], f32, name="rec", tag="rec", bufs=1)
        attnT = []

        for hp in range(NPAIR):
            hA, hB = 2 * hp, 2 * hp + 1
            psoA = psm.tile([P, QLOC], f32, name="psoA", tag="pso", bufs=4)
            psoB = psm.tile([P, QLOC], f32, name="psoB", tag="pso", bufs=4)

            def scores(ti):
                kvs = slice(ti * P, (ti + 1) * P)
                psAB = psm.tile([P, 2 * QLOC], f32, name="psAB", tag="sc")
                nc.tensor.matmul(psAB[:, 0:QLOC], kt[hp][0:DEP, kvs],
                                 qt[hp][0:DEP, :],
                                 start=True, stop=True, tile_position=(0, 0))
                nc.tensor.matmul(psAB[:, QLOC:2 * QLOC], kt[hp][DEP:P, kvs],
                                 qt[hp][DEP:P, :],
                                 start=True, stop=True, tile_position=(64, 0))
                eAB = epl.tile([P, 2 * QLOC], bf16, name="eAB", tag="e")
                nc.scalar.activation(eAB[:], psAB[:], AF.Exp,
                                     bias=mbias[:, ti:ti + 1], scale=0.125)
                return eAB

            eAB = scores(0)
            myfill = pair_fill.get(hp)
            for ti in range(nkt):
                st, fi = (ti == 0), (ti == nkt - 1)
                if hp == 4 and vfill_p4:
                    fn, args = vfill_p4.popleft()
                    fn(*args)
                nc.tensor.matmul(psoA[0:VW, :],
                                 vaug[ti][:, hA * VW:(hA + 1) * VW],
                                 eAB[:, 0:QLOC], start=st, stop=fi)
                nc.tensor.matmul(psoB[0:VW, :],
                                 vaug[ti][:, hB * VW:(hB + 1) * VW],
                                 eAB[:, QLOC:2 * QLOC], start=st, stop=fi)
                if myfill and (ti % 2 == 1 or len(myfill) > (nkt - ti) // 2):
                    fn, args = myfill.popleft()
                    fn(*args)
                if ti + 1 < nkt:
                    eAB = scores(ti + 1)
            while myfill:  # flush this pair's fillers before moving on
                fn, args = myfill.popleft()
                fn(*args)
            if hp == 4:
                # all K/V/Q production done: free wk/wv/xkv SBUF and
                # prefetch the Wo weights into the wq pool's space
                es_kvx.close()
                es_wv.close()
                es_wk.close()
                wo_sb = wqp.tile([P, 2 * KT_D * 512], bf16, name="wo_sb",
                                 tag="wo")
                for h in range(2):
                    for h2 in range(2):
                        nc.scalar.dma_start(
                            out=wo_sb[:, h * 4096 + h2 * 2048:
                                      h * 4096 + (h2 + 1) * 2048].rearrange(
                                "p (k c) -> p k c", c=512),
                            in_=dram3(wo_d, h2 * 512, 4, h * 512, 512))

            # ---- normalization (entirely off the PE queue) ----
            # Custom DVE ISA ops (reciprocal_approx_fast) only work at
            # partition offset 0 and can't read PSUM: stage the two den
            # rows (PSUM partition 64) into SBUF, DMA-shift them to
            # partition 0, recip there, then Pool partition_broadcast.
            nc.vector.tensor_copy(den[DEP:VW, 0:QLOC], psoA[DEP:VW, :])
            nc.vector.tensor_copy(den[DEP:VW, QLOC:2 * QLOC],
                                  psoB[DEP:VW, :])
            nc.sync.dma_start(out=den[0:1, :], in_=den[DEP:VW, :])
            nc.vector.reciprocal_approx_fast(rec[0:1, 0:QLOC],
                                             den[0:1, 0:QLOC])
            nc.vector.reciprocal_approx_fast(rec[0:1, QLOC:2 * QLOC],
                                             den[0:1, QLOC:2 * QLOC])
            rb = nrm.tile([DEP, 2 * QLOC], f32, name="rb", tag="rb", bufs=2)
            nc.gpsimd.partition_broadcast(rb[:], rec[0:1, :])
            at = p_attnT.tile([P, QLOC], bf16, name=f"attnT{hp}", tag="attnT")
            nc.vector.tensor_mul(at[0:DEP, :], psoA[0:DEP, :], rb[:, 0:QLOC])
            tmpB = nrm.tile([DEP, QLOC], bf16, name="tmpB", tag="tmpB",
                            bufs=1)
            nc.vector.tensor_mul(tmpB[:], psoB[0:DEP, :],
                                 rb[:, QLOC:2 * QLOC])
            nc.sync.dma_start(out=at[DEP:P, :], in_=tmpB[:])
            attnT.append(at)
            _dbg_norm = (psoA, psoB, rb, rb)
        if dbg == 'norm':
            psoA_, psoB_, psb_, rb_ = _dbg_norm
            dt1 = cst.tile([VW, 2 * QLOC], f32, name="dt1", tag="dbgn")
            dt2 = cst.tile([VW, 2 * QLOC], f32, name="dt2", tag="dbgn2")
            nc.vector.tensor_copy(dt1[DEP:VW, 0:QLOC], psoA_[DEP:VW, :])
            nc.vector.tensor_copy(dt1[DEP:VW, QLOC:2 * QLOC],
                                  psoB_[DEP:VW, :])
            nc.vector.tensor_copy(dt2[0:DEP, :], psb_[0:DEP, :])
            nc.sync.dma_start(out=out_d[0:1, :], in_=dt1[DEP:VW, 0:QLOC])
            nc.sync.dma_start(out=out_d[1:2, :],
                              in_=dt1[DEP:VW, QLOC:2 * QLOC])
            nc.sync.dma_start(out=out_d[2:3, :], in_=rec[0:1, 0:QLOC])
            nc.sync.dma_start(out=out_d[3:4, :],
                              in_=rec[0:1, QLOC:2 * QLOC])
            nc.sync.dma_start(out=out_d[4:5, :], in_=rec[0:1, 0:QLOC])
            nc.sync.dma_start(out=out_d[5:6, :],
                              in_=rec[0:1, QLOC:2 * QLOC])
            nc.sync.dma_start(out=out_d[128:192, :], in_=dt2[0:DEP, 0:QLOC])
            nc.sync.dma_start(out=out_d[192:256, :],
                              in_=dt2[0:DEP, QLOC:2 * QLOC])
        if dbg == 'attnT':
            dbg_dump(nc, [a[:] for a in attnT], cst)
        if dbg == 'kt':
            dbg_dump(nc, [k[:, 0:QLOC] for k in kt], cst)
        if dbg == 'vaug':
            dbg_dump(nc, [v[:, 0:QLOC] for v in vaug[:MT_D]], cst)
        ep.close()
        es_vaug.close()
        es_ps.close()

        # ---------- Wo + residual + interleaved ln1 sums ----------
        _mark(nc, 'wo')
        es_wo = ExitStack()
        wop = es_wo.enter_context(tc.tile_pool(name="wop", bufs=1,
                                               side="right"))
        xqf_sb = wop.tile([P, KT_D * QLOC], f32, name="xqf_sb", tag="xqf")
        xqfv = xqf_sb[:].rearrange("p (k c) -> p k c", c=QLOC)
        for q4 in range(4):
            nc.sync.dma_start(
                out=xqfv[:, q4 * 2:(q4 + 1) * 2, :],
                in_=dram3(xqf_d, q4 * 256, 2, 0, QLOC))
        es_pp2 = ExitStack()
        pp2 = es_pp2.enter_context(
            tc.tile_pool(name="pp2", bufs=2, space="PSUM", side="right"))

        def ln_make_psums(lnp, tag):
            ssum = lnp.tile([1, QLOC], f32, name=f"ssum{tag}",
                            tag=f"lnsum{tag}", bufs=1)
            ssq = lnp.tile([1, QLOC], f32, name=f"ssq{tag}",
                           tag=f"lnsq{tag}", bufs=1)
            return ssum, ssq

        def ln_accum(ssum, ssq, t, m, last, tag):
            sq = ln_s.tile([P, QLOC], f32r, name=f"sq{tag}", tag="sq", bufs=1)
            nc.scalar.activation(sq[:], t[:].bitcast(f32), AF.Square)
            nc.tensor.matmul(ssum[:], ones[:, 0:1], t[:],
                             start=(m == 0), stop=last)
            nc.tensor.matmul(ssq[:], ones[:, 0:1], sq[:],
                             start=(m == 0), stop=last)

        def ln_norm(ssum, ssq, lnp, src, a_nm, b_nm, out_dtype, tag,
                    opool, otag, extra_bf16=None):
            n = len(src) * P
            mean = ln_s.tile([1, QLOC], f32, name=f"mean{tag}", tag="lns",
                             bufs=3)
            nc.vector.tensor_scalar_mul(mean[:], ssum[:], 1.0 / n)
            m2 = ln_s.tile([1, QLOC], f32, name=f"m2{tag}", tag="lns", bufs=3)
            nc.vector.tensor_mul(m2[:], mean[:], mean[:])
            var = ln_s.tile([1, QLOC], f32, name=f"var{tag}", tag="lns",
                            bufs=3)
            nc.vector.tensor_scalar_mul(var[:], m2[:], -float(n) / (n - 1))
            nc.vector.scalar_tensor_tensor(var[:], ssq[:], 1.0 / (n - 1),
                                           var[:], ALU.mult, ALU.add)
            std = m2  # m2 fully consumed; reuse its slot
            nc.scalar.activation(std[:], var[:], AF.Sqrt)
            stdp = var  # var consumed by sqrt; reuse
            nc.vector.tensor_scalar_add(stdp[:], std[:], EPS)
            rs2 = ln_s.tile([1, 2 * QLOC], f32, name=f"rs2{tag}", tag="rs",
                            bufs=1)
            nc.vector.reciprocal_approx_fast(rs2[0:1, 0:QLOC], stdp[:])
            nc.vector.tensor_mul(rs2[0:1, QLOC:2 * QLOC], mean[:],
                                 rs2[0:1, 0:QLOC])
            rs2r = ln_s.tile([1, 2 * QLOC], f32r, name=f"rs2r{tag}",
                             tag="rsr", bufs=1)
            nc.vector.tensor_copy(rs2r[:], rs2[:])
            bps = lnp.tile([P, 2 * QLOC], f32, name=f"bps{tag}",
                           tag=f"bps{tag}", bufs=1)
            nc.tensor.matmul(bps[:, 0:QLOC], onesr[:], rs2r[0:1, 0:QLOC],
                             start=True, stop=True)
            nc.tensor.matmul(bps[:, QLOC:2 * QLOC], onesr[:],
                             rs2r[0:1, QLOC:2 * QLOC],
                             start=True, stop=True)
            rstd_b = ln_s.tile([P, QLOC], f32, name=f"rstdb{tag}", tag="lnb",
                               bufs=2)
            mrs_b = ln_s.tile([P, QLOC], f32, name=f"mrsb{tag}", tag="lnb",
                              bufs=2)
            nc.vector.tensor_copy(rstd_b[:], bps[:, 0:QLOC])
            nc.vector.tensor_copy(mrs_b[:], bps[:, QLOC:2 * QLOC])
            outs = []
            for m, t in enumerate(src):
                tm = ln_s.tile([P, QLOC], f32r, name=f"tm{tag}", tag="tm",
                               bufs=2)
                nc.vector.tensor_mul(tm[:], t[:].bitcast(f32), rstd_b[:])
                tmf = tm[:].bitcast(f32)
                nc.vector.tensor_sub(tmf, tmf, mrs_b[:])
                if extra_bf16 is not None:
                    ob = extra_bf16[0].tile([P, QLOC], bf16, name=f"o1b{m}",
                                            tag=extra_bf16[1], bufs=MT_D)
                    nc.gpsimd.tensor_scalar(ob[:], tmf, col(a_nm, m),
                                            col(b_nm, m), ALU.mult, ALU.add)
                    extra_bf16[2].append(ob)
                o = opool.tile([P, QLOC], out_dtype, name=f"ln{tag}_{m}",
                               tag=otag)
                nc.scalar.activation(o[:], tmf, AF.Identity,
                                     bias=col(b_nm, m), scale=col(a_nm, m))
                outs.append(o)
            return outs

        lnp1_es = ExitStack()
        lnp1 = lnp1_es.enter_context(
            tc.tile_pool(name="lnp1", bufs=1, space="PSUM", side="right"))
        ssum1, ssq1 = ln_make_psums(lnp1, "1")

        r1 = []
        for h in range(2):
            for ml in range(4):
                m = 4 * h + ml
                ps = pp2.tile([P, QLOC], f32, name="wo_ps", tag="ps2")
                for k in range(KT_D):
                    nc.tensor.matmul(
                        ps[:], wo_sb[:, h * 4096 + k * 512 + ml * 128:
                                     h * 4096 + k * 512 + (ml + 1) * 128],
                        attnT[k][:],
                        start=(k == 0), stop=(k == KT_D - 1))
                t = p_qr.tile([P, QLOC], f32r, name=f"r1_{m}", tag="qr")
                nc.vector.scalar_tensor_tensor(
                    t[:], ps[:], col("bo", m),
                    xqf_sb[:, m * QLOC:(m + 1) * QLOC], ALU.add, ALU.add)
                r1.append(t)
                ln_accum(ssum1, ssq1, t, m, m == MT_D - 1, "1")
        if dbg == 'r1':
            dbg_dump(nc, [t[:] for t in r1], cst)
        es_attnT.close()

        # ---------- ln1 ----------
        _mark(nc, 'ln1')
        out1b = []
        out1 = ln_norm(ssum1, ssq1, lnp1, r1, "a1", "be1", f32, "1",
                       p_qr, "qr", extra_bf16=(p_o1b, "o1b", out1b))
        if dbg == 'out1':
            dbg_dump(nc, [t[:] for t in out1], cst)
        lnp1_es.close()

        # ---------- FFN first linear ----------
        _mark(nc, 'w1')
        p_ht = ctx.enter_context(tc.tile_pool(name="p_ht", bufs=MT_H))
        es_w2p = ExitStack()
        w2p = es_w2p.enter_context(tc.tile_pool(name="w2p", bufs=2))

        def w2_load(mg, kg):
            w2g = w2p.tile([P, 8 * 512], bf16, name=f"w2g{mg}_{kg}",
                           tag="w2")
            nc.sync.dma_start(
                out=w2g[:].rearrange("p (k c) -> p k c", c=512),
                in_=dram3(w2_d, kg * 1024, 8, mg * 512, 512))
            return w2g

        w2pre = {(0, 0): w2_load(0, 0)}  # prefetch during w1
        es_w1 = ExitStack()
        w1p = es_w1.enter_context(tc.tile_pool(name="w1p", bufs=2))
        ht = []
        for g in range(4):
            w1g = w1p.tile([P, KT_D * 1024], bf16, name=f"w1g{g}", tag="w1")
            w1v = w1g[:].rearrange("p (k c) -> p k c", c=1024)
            for h2 in range(2):
                nc.sync.dma_start(
                    out=w1v[:, :, h2 * 512:(h2 + 1) * 512],
                    in_=dram3(w1_d, 0, KT_D, g * 1024 + h2 * 512, 512))
            for mm in range(8):
                m = g * 8 + mm
                ps = pp2.tile([P, QLOC], f32, name="h_ps", tag="ps2")
                for k in range(KT_D):
                    nc.tensor.matmul(
                        ps[:], w1g[:, k * 1024 + mm * 128:
                                   k * 1024 + (mm + 1) * 128],
                        out1b[k][:],
                        start=(k == 0), stop=(k == KT_D - 1))
                t = p_ht.tile([P, QLOC], bf16, name=f"ht{m}", tag="ht")
                nc.scalar.activation(t[:], ps[:], AF.Relu, bias=col("b1", m))
                ht.append(t)
        es_w1.close()
        es_pp2.close()
        es_wo.close()
        es_wq.close()

        # ---------- FFN second linear + interleaved ln2 sums ----------
        _mark(nc, 'w2')
        lnp2_es = ExitStack()
        lnp2 = lnp2_es.enter_context(
            tc.tile_pool(name="lnp2", bufs=1, space="PSUM", side="right"))
        ssum2, ssq2 = ln_make_psums(lnp2, "2")
        r2 = []
        with tc.tile_pool(name="fpp", bufs=1, space="PSUM",
                          side="right") as fpp:
            pend = []  # deferred ln2 sum-matmuls: (m_global, r2 tile)
            for mg in range(2):
                f_ps = [fpp.tile([P, QLOC], f32, name=f"f_ps{mg}_{m}",
                                 tag=f"fps{m}", bufs=1) for m in range(4)]
                for kg in range(4):
                    w2g = w2pre.pop((mg, kg), None)
                    if w2g is None:
                        w2g = w2_load(mg, kg)
                    for k8 in range(8):
                        k = kg * 8 + k8
                        for m in range(4):
                            nc.tensor.matmul(
                                f_ps[m][:], w2g[:, k8 * 512 + m * 128:
                                                k8 * 512 + (m + 1) * 128],
                                ht[k][:],
                                start=(k == 0), stop=(k == MT_H - 1))
                    if kg == 0 and pend:
                        # previous group's ln2 sums (inputs ready by now)
                        for mi, t in pend:
                            ln_accum(ssum2, ssq2, t, mi, mi == MT_D - 1, "2")
                        pend = []
                for m in range(4):
                    mi = mg * 4 + m
                    t = p_kt.tile([P, QLOC], f32r, name=f"r2_{mi}", tag="kt")
                    nc.vector.scalar_tensor_tensor(
                        t[:], f_ps[m][:], col("b2", mi), out1[mi][:],
                        ALU.add, ALU.add)
                    r2.append(t)
                    pend.append((mi, t))
            for mi, t in pend:
                ln_accum(ssum2, ssq2, t, mi, mi == MT_D - 1, "2")
            if dbg == 'ht':
                dbg_dump(nc, [t[:] for t in ht[:MT_D]], cst)
            if dbg == 'r2':
                dbg_dump(nc, [t[:] for t in r2], cst)

            # ---------- ln2 + output ----------
            _mark(nc, 'ln2')
            out2 = ln_norm(ssum2, ssq2, lnp2, r2, "a2", "be2", f32, "2",
                           p_kt, "kt")
        es_w2p.close()
        lnp2_es.close()
        if dbg is None:
            for m in range(MT_D):
                nc.sync.dma_start(out=out_d[m * P:(m + 1) * P, :],
                                  in_=out2[m][:])
        _mark(nc, 'end')

    nc.compile()
    return nc


_cache = {}


def _get_nc(nkv):
    import os
    dbg = os.environ.get("KDBG") or None
    key = (nkv, dbg)
    if key not in _cache:
        _cache[key] = build(nkv, dbg)
    return _cache[key]


def kernel(x, mask, Wq, bq, Wk, bk, Wv, bv, Wo, bo, alpha1, beta1,
           W1, b1, W2, b2, alpha2, beta2):
    x = np.asarray(x, np.float32)
    mask = np.asarray(mask)

    idx = [np.nonzero(np.asarray(mask[b]) == 0)[0] for b in range(B)]
    nkv = ((max(len(i) for i in idx) + P - 1) // P) * P
    nkv = max(nkv, P)
    nkt = nkv // P

    nc = _get_nc(nkv)

    def colmaj(v, mt):
        return np.asarray(v, np.float32).reshape(mt, P).T

    bo_eff = (np.asarray(bo, np.float32)
              + np.asarray(bv, np.float32) @ np.asarray(Wo, np.float32))

    cb = np.concatenate([
        colmaj(bq, MT_D), colmaj(bk, MT_D), colmaj(bo_eff, MT_D),
        colmaj(b1, MT_H), colmaj(b2, MT_D), colmaj(alpha1, MT_D),
        colmaj(beta1, MT_D), colmaj(alpha2, MT_D), colmaj(beta2, MT_D),
    ], axis=1)
    assert cb.shape == (P, 96)

    bf = ml_dtypes.bfloat16
    common = {
        "wq": np.ascontiguousarray(Wq, dtype=bf),
        "wk": np.ascontiguousarray(Wk, dtype=bf),
        "wv": np.ascontiguousarray(Wv, dtype=bf),
        "wo": np.ascontiguousarray(Wo, dtype=bf),
        "w1": np.ascontiguousarray(W1, dtype=bf),
        "w2": np.ascontiguousarray(W2, dtype=bf),
        "cb": np.ascontiguousarray(cb),
        "cone1": np.ones((P, 1), np.float32),
        "crow": np.ones((1, P), np.float32),
    }

    per_batch = []
    for b in range(B):
        ib = idx[b]
        xkv = np.zeros((D, nkv), bf)
        xkv[:, :len(ib)] = x[b][ib].T.astype(bf)
        mb = np.zeros(nkv, np.float32)
        mb[len(ib):] = PADBIAS
        mb = np.ascontiguousarray(mb.reshape(nkt, P).T)
        per_batch.append((xkv, mb, np.ascontiguousarray(x[b].T)))

    in_maps = []
    for c in range(NCORES):
        b = c // 4
        qoff = (c % 4) * QLOC
        xkv, mb, xT = per_batch[b]
        m = dict(common)
        m["xq"] = np.ascontiguousarray(xT[:, qoff:qoff + QLOC].astype(bf))
        m["xqf"] = np.ascontiguousarray(xT[:, qoff:qoff + QLOC])
        m["xkv"] = xkv
        m["mb"] = mb
        in_maps.append(m)

    res = None
    for attempt in range(3):
        try:
            res = run_bass_kernel_spmd(nc, in_maps, list(range(NCORES)))
            break
        except Exception:
            if attempt == 2:
                raise

    out = np.empty((B, S, D), np.float32)
    for c in range(NCORES):
        b = c // 4
        qoff = (c % 4) * QLOC
        out[b, qoff:qoff + QLOC, :] = res.results[c]["out"].T
    return out
